# revision 3
# baseline (speedup 1.0000x reference)
"""Trainium2 Bass kernel for nn_DecoderModule (dense transformer decoder layer).

Distribution (8 NeuronCores, tensor-parallel attention + row-parallel FFN):
  - Each core owns 2 of the 16 heads: computes Q/K/V + causal attention for
    its heads over the full sequence (T=2048), normalized head outputs kept
    TRANSPOSED [head_dim, T] in bf16.
  - One AllToAll (0.5 MB/rank) redistributes head outputs so core c holds
    ALL 16 heads restricted to its 256-row block.
  - Pool projection, residual+LN, and the full FFN then run row-parallel on
    the core's 256 rows; host concatenates the 8 row blocks.

v4 = v2 (packed 2-blob inputs, deferred consts, pipelined pool/LN1, split
LN2 stats) plus:
  - ei=0 slices of wq/wk/wv load first as small DMAs so the first QKV
    matmul starts ~1us in.
  - Late consts (mask/poolw/xr/biases) issue on the gpsimd DMA queue so
    they never contend with the SP xt-tile stream.
  - Final LN2 apply split across Scalar+Vector engines.
"""

import sys

sys.path.insert(0, "/opt/trn_rl_repo")

import numpy as np  # noqa: E402
import ml_dtypes  # noqa: E402

import concourse.bass as bass  # noqa: E402
import concourse.tile as tile  # noqa: E402
from concourse import mybir  # noqa: E402
from concourse.bass_utils import run_bass_kernel_spmd  # noqa: E402
from concourse.masks import make_identity  # noqa: E402

T, E, H, D, F = 2048, 1024, 16, 64, 4096
NCORES = 8
HPC = H // NCORES          # heads per core = 2
TB = T // NCORES           # rows per core = 256
EPS = 1e-5

F32 = mybir.dt.float32
BF16 = mybir.dt.bfloat16
AF = mybir.ActivationFunctionType
Alu = mybir.AluOpType
BF16NP = ml_dtypes.bfloat16

# --- packed blob layouts (element offsets) -------------------------------
OFF_XT = 0                        # [E=1024, T=2048] x^T, bf16
OFF_WQ = OFF_XT + E * T           # [E, 128]  2 heads' W_Q columns
OFF_WK = OFF_WQ + E * 128
OFF_WV = OFF_WK + E * 128
OFF_POOLW = OFF_WV + E * 128      # [E, E]
OFF_L1W = OFF_POOLW + E * E       # [E, F]
OFF_L2W = OFF_L1W + E * F         # [F, E]
OFF_MASK = OFF_L2W + F * E        # [4, 128, 512] causal diag-block masks
NB = OFF_MASK + 4 * 128 * 512

OFF_XR = 0                        # [TB=256, E] row block of x, f32
OFF_L1B = OFF_XR + TB * E         # [F]
OFF_L2B = OFF_L1B + F             # [E]
OFF_GAM = OFF_L2B + E             # [1]
OFF_BETA = OFF_GAM + 1            # [E]
NF = OFF_BETA + E


def _split_waits(nc, limit=1):
    """This walrus build rejects >1 sync-wait per instruction. Hoist extra
    waits onto engine-native nops inserted immediately before the owner."""
    tail_bb = nc.cur_bb.bb

    def make_carrier(engine, wait):
        inst_obj = nc.engines[engine].nop(nofuse=True, hint="waitsplit")
        mi = inst_obj.ins
        tl = tail_bb.instructions
        assert tl[-1] is mi
        tl.pop()
        if mi.sync_info is None:
            mi.sync_info = mybir.SyncInfo(on_wait=[wait], on_update=[])
        else:
            mi.sync_info.on_wait = [wait]
        return mi

    n = 0
    for bb in nc.main_func.blocks:
        il = bb.instructions
        out = []
        for ins in il:
            si = getattr(ins, "sync_info", None)
            waits = list(si.on_wait) if (si and si.on_wait) else []
            if len(waits) > limit:
                extra, keep = waits[:-limit], waits[-limit:]
                for w in extra:
                    out.append(make_carrier(ins.engine, w))
                    n += 1
                si.on_wait = keep
            out.append(ins)
        il[:] = out
    return n


def build_nc():
    nc = bass.Bass()

    bufb = nc.declare_dram_parameter("bufb", [NB], BF16, isOutput=False)
    buff = nc.declare_dram_parameter("buff", [NF], F32, isOutput=False)
    out = nc.declare_dram_parameter("out", [TB, E], F32, isOutput=True)

    with tile.TileContext(nc) as tc:
        _body(tc, bufb, buff, out)

    _split_waits(nc)
    return nc


def _bv(t, off, ap):
    return bass.AP(tensor=t, offset=off, ap=ap)


def _body(tc, bufb, buff, out):
    nc = tc.nc
    dma = nc.sync.dma_start
    gdma = nc.gpsimd.dma_start

    from contextlib import ExitStack
    ctx = ExitStack()
    const = ctx.enter_context(tc.tile_pool(name="const", bufs=1))
    sb = ctx.enter_context(tc.tile_pool(name="work", bufs=2))
    dram = ctx.enter_context(tc.tile_pool(name="dram", bufs=1, space="DRAM"))

    # ---- QKV weights: ei=0 slice first (tiny DMAs) so PE starts at ~1us --
    wq_s = const.tile([128, 8, 128], BF16)
    wk_s = const.tile([128, 8, 128], BF16)
    wv_s = const.tile([128, 8, 128], BF16)
    for w_s, off in ((wq_s, OFF_WQ), (wk_s, OFF_WK), (wv_s, OFF_WV)):
        dma(out=w_s[:, 0, :], in_=_bv(bufb, off, [[128, 128], [1, 128]]))
    for w_s, off in ((wq_s, OFF_WQ), (wk_s, OFF_WK), (wv_s, OFF_WV)):
        dma(out=w_s[:, 1:8, :],
            in_=_bv(bufb, off + 128 * 128, [[128, 128], [128 * 128, 7], [1, 128]]))

    # late-needed consts on the gpsimd DMA queue (never block the xt stream)
    mask_s = const.tile([128, 4, 512], BF16)
    gdma(out=mask_s[:], in_=_bv(bufb, OFF_MASK, [[512, 128], [128 * 512, 4], [1, 512]]))
    xr_s = const.tile([128, 2, E], F32)
    gdma(out=xr_s[:], in_=_bv(buff, OFF_XR, [[E, 128], [128 * E, 2], [1, E]]))
    poolw_s = const.tile([128, 8, E], BF16)
    gdma(out=poolw_s[:], in_=_bv(bufb, OFF_POOLW, [[E, 128], [128 * E, 8], [1, E]]))
    l1b_s = const.tile([128, 32], F32)
    gdma(out=l1b_s[:], in_=_bv(buff, OFF_L1B, [[1, 128], [128, 32]]))
    beta_s = const.tile([128, E], F32)
    gdma(out=beta_s[:], in_=_bv(buff, OFF_BETA, [[0, 128], [1, E]]))
    l2b_s = const.tile([128, E], F32)
    gdma(out=l2b_s[:], in_=_bv(buff, OFF_L2B, [[0, 128], [1, E]]))
    gam_s = const.tile([128, 1], F32)
    gdma(out=gam_s[:], in_=_bv(buff, OFF_GAM, [[0, 128], [1, 1]]))

    eps_s = const.tile([128, 1], F32)
    nc.vector.memset(eps_s[:], EPS)
    identf = const.tile([128, 128], F32)
    make_identity(nc, identf[:])
    identb = const.tile([128, 128], BF16)
    make_identity(nc, identb[:])

    qT = const.tile([128, 4, 512], BF16)      # [d2 | tt, t]
    kT = const.tile([128, 4, 512], BF16)
    vp = const.tile([128, 16, 130], BF16)     # [k | ki, (v0|1|v1|1)]
    hnT = const.tile([128, T], BF16)          # normalized headsT, both heads
    y1 = const.tile([128, 2, E], F32)         # x + attn  (my 256 rows)
    h1 = const.tile([128, 2, E], F32)         # LN1 out
    hT = const.tile([128, 8, 256], BF16)      # h transposed [e, t]
    relu_s = const.tile([128, 32, 256], BF16)  # relu(l1) transposed [f, t]
    y2 = const.tile([128, 2, E], F32)
    out_s = const.tile([128, 2, E], F32)

    nc.vector.memset(vp[:, :, 64:65], 1.0)
    nc.vector.memset(vp[:, :, 129:130], 1.0)

    # ---- phase B: QKV ----------------------------------------------------
    with tc.tile_pool(name="psB", bufs=2, space="PSUM") as psB, \
         tc.tile_pool(name="psV", bufs=1, space="PSUM") as psV, \
         tc.tile_pool(name="xts", bufs=4) as xts:
        for tt in range(4):
            ps_q = psB.tile([128, 512], F32, tag="q")
            ps_k = psB.tile([128, 512], F32, tag="k")
            ps_v = [psV.tile([128, 128], F32, tag=f"v{s}", name=f"v{s}")
                    for s in range(4)]
            for ei in range(8):
                xt_t = xts.tile([128, 512], BF16, tag="xt")
                dma(out=xt_t[:],
                    in_=_bv(bufb, OFF_XT + 128 * ei * T + 512 * tt,
                            [[T, 128], [1, 512]]))
                st, sp = (ei == 0), (ei == 7)
                nc.tensor.matmul(ps_q[:], wq_s[:, ei, :], xt_t[:], start=st, stop=sp)
                nc.tensor.matmul(ps_k[:], wk_s[:, ei, :], xt_t[:], start=st, stop=sp)
                for s in range(4):
                    nc.tensor.matmul(ps_v[s][:],
                                     xt_t[:, 128 * s:128 * (s + 1)],
                                     wv_s[:, ei, :], start=st, stop=sp)
            nc.vector.tensor_copy(qT[:, tt, :], ps_q[:])
            nc.vector.tensor_copy(kT[:, tt, :], ps_k[:])
            for s in range(4):
                ki = 4 * tt + s
                nc.vector.tensor_copy(vp[:, ki, 0:64], ps_v[s][:, 0:64])
                nc.vector.tensor_copy(vp[:, ki, 65:129], ps_v[s][:, 64:128])

    # ---- phase C: attention ---------------------------------------------
    a2a_in = dram.tile([8, 128, 256], BF16)
    a2a_out = dram.tile([8, 128, 256], BF16)
    kTf = kT[:].rearrange("p tt t -> p (tt t)")
    with tc.tile_pool(name="psC", bufs=1, space="PSUM") as psC, \
         tc.tile_pool(name="psS", bufs=2, space="PSUM") as psS, \
         tc.tile_pool(name="att", bufs=4) as att, \
         tc.tile_pool(name="psT", bufs=2, space="PSUM") as psT:
        for qt in range(4):
            rows = [att.tile([128, 128], BF16, tag=f"rows{s}", name=f"rows{s}") for s in range(4)]
            for hh in range(2):
                hb = 64 * hh
                ps_av = [psC.tile([128, 65], F32, tag=f"av{s}", name=f"av{s}") for s in range(4)]
                nki = 4 * qt + 4
                for ki in range(nki):
                    ps_s = psS.tile([128, 512], F32, tag="sc")
                    nc.tensor.matmul(
                        ps_s[:],
                        kTf[hb:hb + 64, 128 * ki:128 * (ki + 1)],
                        qT[hb:hb + 64, qt, :], start=True, stop=True)
                    ex = att.tile([128, 512], BF16, tag="exp")
                    nc.scalar.activation(ex[:], ps_s[:], AF.Exp)
                    r = ki - 4 * qt
                    if r >= 0:
                        # only the diagonal 128x128 sub-block is partial;
                        # sub-blocks s<r are skipped below, s>r fully valid
                        blk = slice(128 * r, 128 * (r + 1))
                        nc.vector.tensor_mul(ex[:, blk], ex[:, blk],
                                             mask_s[:, r, blk])
                    for s in range(max(r, 0), 4):
                        nc.tensor.matmul(
                            ps_av[s][:], ex[:, 128 * s:128 * (s + 1)],
                            vp[:, ki, 65 * hh:65 * hh + 65],
                            start=(ki == 0), stop=(ki == 4 * qt + s))
                for s in range(4):
                    rec = att.tile([128, 1], F32, tag="rec")
                    nc.vector.reciprocal(rec[:], ps_av[s][:, 64:65])
                    nc.vector.tensor_scalar_mul(
                        out=rows[s][:, hb:hb + 64], in0=ps_av[s][:, 0:64],
                        scalar1=rec[:])
            for s in range(4):
                qg = 4 * qt + s
                pt = psT.tile([128, 128], BF16, tag="tp")
                nc.tensor.transpose(pt[:], rows[s][:], identb[:])
                nc.vector.tensor_copy(hnT[:, 128 * qg:128 * (qg + 1)], pt[:])

    for j in range(8):
        dma(out=a2a_in[j], in_=hnT[:, 256 * j:256 * (j + 1)])
    nc.gpsimd.collective_compute(
        "AllToAll", Alu.bypass, replica_groups=[list(range(NCORES))],
        ins=[a2a_in[:].opt()], outs=[a2a_out[:].opt()])
    heads_sb = const.tile([128, 8, 256], BF16)
    for j in range(8):
        dma(out=heads_sb[:, j, :], in_=a2a_out[j])

    # ---- phase D+E: pool + residual + LN1 + transpose, per row-half ------
    with tc.tile_pool(name="psD", bufs=2, space="PSUM") as psD, \
         tc.tile_pool(name="psE", bufs=2, space="PSUM") as psE:
        for qs in range(2):
            for eh in range(2):
                ps_p = psD.tile([128, 512], F32, tag="pool")
                for j in range(8):
                    nc.tensor.matmul(
                        ps_p[:], heads_sb[:, j, 128 * qs:128 * (qs + 1)],
                        poolw_s[:, j, 512 * eh:512 * (eh + 1)],
                        start=(j == 0), stop=(j == 7))
                nc.vector.tensor_add(y1[:, qs, 512 * eh:512 * (eh + 1)],
                                     xr_s[:, qs, 512 * eh:512 * (eh + 1)],
                                     ps_p[:])
            _ln(nc, sb, y1[:, qs, :], h1[:, qs, :], gam_s, beta_s, eps_s)
            for et in range(8):
                pt = psE.tile([128, 128], F32, tag="tp")
                nc.tensor.transpose(pt[:], h1[:, qs, 128 * et:128 * (et + 1)],
                                    identf[:])
                nc.vector.tensor_copy(hT[:, et, 128 * qs:128 * (qs + 1)], pt[:])

    # ---- phase F: FFN l1 -------------------------------------------------
    with tc.tile_pool(name="psF", bufs=2, space="PSUM") as psF, \
         tc.tile_pool(name="l1s", bufs=8) as l1s:
        for fg in range(8):
            ps_f = [psF.tile([128, 256], F32, tag=f"l1_{s}", name=f"l1_{s}") for s in range(4)]
            for et in range(8):
                l1t = l1s.tile([128, 512], BF16, tag="l1w")
                dma(out=l1t[:],
                    in_=_bv(bufb, OFF_L1W + 128 * et * F + 512 * fg,
                            [[F, 128], [1, 512]]))
                for s in range(4):
                    nc.tensor.matmul(ps_f[s][:], l1t[:, 128 * s:128 * (s + 1)],
                                     hT[:, et, :], start=(et == 0), stop=(et == 7))
            for s in range(4):
                ft = 4 * fg + s
                nc.scalar.activation(relu_s[:, ft, :], ps_f[s][:], AF.Relu,
                                     bias=l1b_s[:, ft:ft + 1])

    # ---- phase G: FFN l2 + residual + LN2, stats pipelined per E-half ----
    stats2 = [sb.tile([128, 2, 6], F32, tag=f"ln2_st{qs}", name=f"ln2st{qs}")
              for qs in range(2)]
    with tc.tile_pool(name="psG", bufs=2, space="PSUM") as psG, \
         tc.tile_pool(name="l2s", bufs=8) as l2s:
        for eh in range(2):
            ps_o = [psG.tile([128, 512], F32, tag=f"l2_{qs}", name=f"l2_{qs}") for qs in range(2)]
            for ft in range(32):
                l2t = l2s.tile([128, 512], BF16, tag="l2w")
                dma(out=l2t[:],
                    in_=_bv(bufb, OFF_L2W + 128 * ft * E + 512 * eh,
                            [[E, 128], [1, 512]]))
                for qs in range(2):
                    nc.tensor.matmul(ps_o[qs][:],
                                     relu_s[:, ft, 128 * qs:128 * (qs + 1)],
                                     l2t[:], start=(ft == 0), stop=(ft == 31))
            for qs in range(2):
                sl = slice(512 * eh, 512 * (eh + 1))
                nc.vector.tensor_add(y2[:, qs, sl], h1[:, qs, sl], ps_o[qs][:])
                nc.vector.tensor_add(y2[:, qs, sl], y2[:, qs, sl],
                                     l2b_s[:, sl])
                nc.vector.bn_stats(out=stats2[qs][:, eh, :], in_=y2[:, qs, sl])

    # final LN2 aggregation + apply; the (y-mean)*scl pass for qs=0 runs on
    # the Scalar engine in parallel with qs=1's Vector-engine pass.
    for qs in range(2):
        mv = sb.tile([128, 2], F32, tag="ln2_mv", name=f"ln2mv{qs}")
        nc.vector.bn_aggr(out=mv[:], in_=stats2[qs][:])
        std = sb.tile([128, 1], F32, tag="ln2_std")
        nc.scalar.activation(std[:], mv[:, 1:2], AF.Sqrt, bias=eps_s[:])
        rstd = sb.tile([128, 1], F32, tag="ln2_rstd")
        nc.vector.reciprocal(rstd[:], std[:])
        scl = sb.tile([128, 1], F32, tag="ln2_scl", name=f"ln2scl{qs}")
        nc.vector.tensor_mul(scl[:], rstd[:], gam_s[:])
        if qs == 0:
            negms = sb.tile([128, 1], F32, tag="ln2_negms")
            nc.vector.tensor_mul(negms[:], mv[:, 0:1], scl[:])
            nc.scalar.mul(negms[:], negms[:], -1.0)
            nc.scalar.activation(out_s[:, qs, :], y2[:, qs, :], AF.Identity,
                                 bias=negms[:], scale=scl[:])
        else:
            nc.vector.tensor_scalar(
                out=out_s[:, qs, :], in0=y2[:, qs, :], scalar1=mv[:, 0:1],
                scalar2=scl[:], op0=Alu.subtract, op1=Alu.mult,
            )
        nc.vector.tensor_add(out_s[:, qs, :], out_s[:, qs, :], beta_s[:])
        dma(out=_bv(out, 128 * qs * E, [[E, 128], [1, E]]),
            in_=out_s[:, qs, :])

    ctx.close()


def _ln(nc, sb, y_ap, out_ap, gam_s, beta_s, eps_s):
    """LayerNorm over the free dim (1024) of y_ap [128, 1024] -> out_ap."""
    stats = sb.tile([128, 2, 6], F32, tag="ln_stats")
    yv = y_ap.rearrange("p (s d) -> p s d", s=2)
    for s in range(2):
        nc.vector.bn_stats(out=stats[:, s, :], in_=yv[:, s, :])
    mv = sb.tile([128, 2], F32, tag="ln_mv")
    nc.vector.bn_aggr(out=mv[:], in_=stats[:])
    std = sb.tile([128, 1], F32, tag="ln_std")
    nc.scalar.activation(std[:], mv[:, 1:2], AF.Sqrt, bias=eps_s[:])
    rstd = sb.tile([128, 1], F32, tag="ln_rstd")
    nc.vector.reciprocal(rstd[:], std[:])
    scl = sb.tile([128, 1], F32, tag="ln_scl")
    nc.vector.tensor_mul(scl[:], rstd[:], gam_s[:])
    nc.vector.tensor_scalar(
        out=out_ap, in0=y_ap, scalar1=mv[:, 0:1], scalar2=scl[:],
        op0=Alu.subtract, op1=Alu.mult,
    )
    nc.vector.tensor_add(out_ap, out_ap, beta_s[:])


_NC = None


def _get_nc():
    global _NC
    if _NC is None:
        _NC = build_nc()
    return _NC


def make_in_maps(x, wq, wk, wv, pool_w, l1_w, l1_b, l2_w, l2_b, gamma, beta):
    x = np.asarray(x, np.float32)
    wq = np.asarray(wq, np.float32) / np.sqrt(np.float32(D))
    wk = np.asarray(wk, np.float32)
    wv = np.asarray(wv, np.float32)
    xt = np.ascontiguousarray(x.T).astype(BF16NP)
    poolw = np.ascontiguousarray(np.asarray(pool_w, np.float32)).astype(BF16NP)
    l1wn = np.ascontiguousarray(np.asarray(l1_w, np.float32)).astype(BF16NP)
    l2wn = np.ascontiguousarray(np.asarray(l2_w, np.float32)).astype(BF16NP)
    rr, pp, ff = np.meshgrid(np.arange(4), np.arange(128), np.arange(512),
                             indexing="ij")
    maskb = ((128 * rr + pp) <= ff).astype(BF16NP)

    base_b = np.empty(NB, BF16NP)
    base_b[OFF_XT:OFF_XT + E * T] = xt.ravel()
    base_b[OFF_POOLW:OFF_POOLW + E * E] = poolw.ravel()
    base_b[OFF_L1W:OFF_L1W + E * F] = l1wn.ravel()
    base_b[OFF_L2W:OFF_L2W + F * E] = l2wn.ravel()
    base_b[OFF_MASK:NB] = maskb.ravel()

    base_f = np.empty(NF, np.float32)
    base_f[OFF_L1B:OFF_L1B + F] = np.asarray(l1_b, np.float32).ravel()
    base_f[OFF_L2B:OFF_L2B + E] = np.asarray(l2_b, np.float32).ravel()
    base_f[OFF_GAM] = np.asarray(gamma, np.float32).ravel()[0]
    base_f[OFF_BETA:OFF_BETA + E] = np.asarray(beta, np.float32).ravel()

    in_maps = []
    for c in range(NCORES):
        bb = base_b.copy()
        wqc = np.concatenate([wq[2 * c], wq[2 * c + 1]], axis=1).astype(BF16NP)
        wkc = np.concatenate([wk[2 * c], wk[2 * c + 1]], axis=1).astype(BF16NP)
        wvc = np.concatenate([wv[2 * c], wv[2 * c + 1]], axis=1).astype(BF16NP)
        bb[OFF_WQ:OFF_WQ + E * 128] = wqc.ravel()
        bb[OFF_WK:OFF_WK + E * 128] = wkc.ravel()
        bb[OFF_WV:OFF_WV + E * 128] = wvc.ravel()
        ff_ = base_f.copy()
        ff_[OFF_XR:OFF_XR + TB * E] = x[TB * c:TB * (c + 1)].ravel()
        in_maps.append({"bufb": bb, "buff": ff_})
    return in_maps


def kernel(**inputs):
    nc = _get_nc()
    in_maps = make_in_maps(**inputs)
    last = None
    for attempt in range(3):
        try:
            res = run_bass_kernel_spmd(nc, in_maps, list(range(NCORES)))
            return np.concatenate(
                [np.asarray(res.results[c]["out"]) for c in range(NCORES)], axis=0)
        except Exception as e:  # transient axon/device desync — retry
            last = e
            import time as _time
            _time.sleep(5)
    raise last
